# revision 2
# baseline (speedup 1.0000x reference)
"""GCN layer on 8 TRN2 NeuronCores — two-pass kappa-dense shuffle.

out = segment_sum(edge_vals[:,None] * (X @ W)[edge_col], edge_row, N)

The v0 baseline gathered one 256B row per edge via gpsimd dma_gather; Q7
descriptor generation (~8ns/edge, serial) was 84% of its kernel time. Here
edges are routed in two passes with NO per-edge descriptors except a ~5%
overflow remainder:

Pass 1 (per 128-src tile t, in groups of G=4): XW_t computed on-chip (fused,
no XW roundtrip); a host-shipped 0/1 fp8 routing matrix R_t scatters each
edge's XW row via TensorE (M = R_t^T @ XW_t) into a dense per-(dest-window,
tile) kappa=8 slot layout; edge values fold in via one VectorE multiply per
group (cast+scale PSUM->SBUF). Rows land in DRAM with 5 strided DMAs per
group: M1REG[t][w*8+j] plus a fixed-capacity (28) per-tile overflow region.

Pass 2 (per dest-window w): ONE strided DMA pulls the window's [391*8, 64]
slot rows onto partitions; ONE batched VectorE is_equal builds all 25 one-hot
S chunks from rowloc metadata (dead slots rowloc=-1); TensorE accumulates
S^T @ rows into the window PSUM. Overflow edges ride the old dma_gather path
at 1/20 scale (~6k descriptors vs 115k).
"""

from contextlib import ExitStack

import ml_dtypes
import numpy as np

import concourse.bacc as bacc
import concourse.bass as bass
import concourse.mybir as mybir
import concourse.tile as tile
from concourse._compat import get_trn_type
from concourse.bass_utils import run_bass_kernel_spmd

N_NODES = 50000
N_EDGES = 800000
F_IN = 256
F_OUT = 64
N_CORES = 8
SHARD = N_NODES // N_CORES  # 6250
WIN = 128
NW = (SHARD + WIN - 1) // WIN  # 49
NT = (N_NODES + 127) // 128  # 391
KAP = 8
SLOT_T = 448  # slots per src tile: 392 regular + overflow + pad (3.5 chunks)
CHW = (128, 128, 128, 64)  # R chunk widths
NCH = len(CHW)
REG_SLOTS = NW * KAP  # 392
MCOLS = (NT * KAP + 127) // 128  # 25 merge chunks per window
NT_PAD = MCOLS * 16  # 400 tiles incl. 9 zero pad tiles for the merge AP
BF16 = ml_dtypes.bfloat16
FP8 = ml_dtypes.float8_e4m3

SLAB = 4096  # pass-1 node columns per XT slab DMA
RG = 16  # src tiles per R slab load
G = 4  # src tiles per pass-1 group (PSUM budget bound)
CH_OVF = 25  # overflow gather tiles per dma_gather call

TRACE = False
DEBUG_M1 = False
LAST_RESULTS = None
LAST_META = None
LAST_IN_MAPS = None


def _install_ntff_hook():
    import sys
    import types

    try:
        import antenv.axon_hooks  # noqa: F401

        return True
    except ImportError:
        pass
    try:
        import antenv
        from trn_agent_boot.trn_boot import _ntff_profile_via_ctypes

        mod = types.ModuleType("antenv.axon_hooks")
        mod._hook = None

        def set_axon_ntff_profile_hook(h):
            mod._hook = h

        def get_axon_ntff_profile_hook():
            return mod._hook

        mod.set_axon_ntff_profile_hook = set_axon_ntff_profile_hook
        mod.get_axon_ntff_profile_hook = get_axon_ntff_profile_hook
        sys.modules["antenv.axon_hooks"] = mod
        antenv.axon_hooks = mod
        hook = _ntff_profile_via_ctypes("/opt/axon/libaxon_pjrt.so")
        if hook is not None:
            set_axon_ntff_profile_hook(hook)
        return hook is not None
    except Exception as e:
        print(f"ntff hook install failed: {e}")
        return False


def _wrap16(stream_i16, n_tiles, pad=0):
    n = n_tiles * 128
    w = np.zeros((128, max(n // 16, 1)), dtype=np.int16)
    s = np.full(n, pad, dtype=np.int16)
    s[: len(stream_i16)] = stream_i16
    blk = s.reshape(n // 16, 16).T
    for g in range(8):
        w[g * 16 : (g + 1) * 16, :] = blk
    return w


def _prep(X, W, edge_row, edge_col, edge_vals):
    XT = np.ascontiguousarray(X.T).astype(BF16)
    Wb = np.ascontiguousarray(W).astype(BF16)

    core = edge_row // SHARD
    percore = []
    ovf_cnt = np.zeros((N_CORES, NT), dtype=np.int64)
    ovf_win_cnt = np.zeros((N_CORES, NW), dtype=np.int64)
    for p in range(N_CORES):
        m = core == p
        r = edge_row[m].astype(np.int64) - p * SHARD
        c = edge_col[m].astype(np.int64)
        v = edge_vals[m].astype(np.float64)
        t = c // 128
        sloc = c % 128
        w = r // WIN
        rloc = r % WIN
        order = np.lexsort((w, t))
        t, sloc, w, rloc, v = t[order], sloc[order], w[order], rloc[order], v[order]
        seg_id = t * NW + w
        cnts = np.bincount(seg_id, minlength=NT * NW)
        starts = np.concatenate([[0], np.cumsum(cnts)])
        rank = np.arange(len(t)) - starts[seg_id]
        is_ovf = rank >= KAP
        percore.append(
            dict(t=t, sloc=sloc, w=w, rloc=rloc, v=v, rank=rank, is_ovf=is_ovf)
        )
        ovf_cnt[p] = np.bincount(t[is_ovf], minlength=NT)
        ovf_win_cnt[p] = np.bincount(w[is_ovf], minlength=NW)

    ovf_fix = max(1, int(ovf_cnt.max()))  # fixed per-tile overflow capacity
    assert ovf_fix <= SLOT_T - REG_SLOTS - 8, ovf_fix
    n_ovf_rows = NT * ovf_fix
    assert n_ovf_rows < 32000, n_ovf_rows
    T_ovf = np.maximum(1, -(-ovf_win_cnt.max(axis=0) // 128))
    ovf_tile_starts = np.concatenate([[0], np.cumsum(T_ovf)])
    T_OVF_TOT = int(T_ovf.sum())
    assert T_OVF_TOT <= NW * CH_OVF

    in_maps = []
    for p in range(N_CORES):
        d = percore[p]
        t, sloc, w, rloc, v, rank, is_ovf = (
            d["t"], d["sloc"], d["w"], d["rloc"], d["v"], d["rank"], d["is_ovf"],
        )
        ne = len(t)
        slot = np.zeros(ne, dtype=np.int64)
        reg = ~is_ovf
        # slot map: w<48 -> (w//16)*128 + (w%16)*8 + j ; w=48 -> 384 + j.
        # Makes the M1[w][j][t] DRAM offset linear in the PSUM partition.
        wr, jr = w[reg], rank[reg]
        slot[reg] = np.where(
            wr < 48, (wr // 16) * 128 + (wr % 16) * 8 + jr, 384 + jr
        )
        ovf_idx_in_tile = np.zeros(ne, dtype=np.int64)
        for ti in np.unique(t[is_ovf]):
            mm = is_ovf & (t == ti)
            ovf_idx_in_tile[mm] = np.arange(mm.sum())
        slot[is_ovf] = 384 + 8 + ovf_idx_in_tile[is_ovf]

        r8 = np.zeros((128, NT * SLOT_T), dtype=FP8)
        valm = np.zeros((128, NT * NCH), dtype=BF16)
        gs = t * SLOT_T + slot
        r8[sloc, gs] = 1.0
        valm[slot % 128, t * NCH + slot // 128] = v.astype(BF16)

        # merge stream position within window: u = j*400 + t, landed
        # partition-major (partition u//25, col u%25) for fat DMA pieces
        rloc_reg = np.full((128, NW * MCOLS), -1.0, dtype=np.float32)
        u = rank[reg] * NT_PAD + t[reg]
        rloc_reg[u // MCOLS, w[reg] * MCOLS + u % MCOLS] = rloc[reg]

        # compact overflow rows at fixed per-tile stride
        ovf_row = t[is_ovf] * ovf_fix + ovf_idx_in_tile[is_ovf]
        ovf_w = w[is_ovf]
        ovf_rloc = rloc[is_ovf]
        o2 = np.argsort(ovf_w, kind="stable")
        ovf_row, ovf_w, ovf_rloc = ovf_row[o2], ovf_w[o2], ovf_rloc[o2]
        cols_stream = np.full(T_OVF_TOT * 128, n_ovf_rows, dtype=np.int16)
        rloc_ovf = np.full((128, T_OVF_TOT), -1.0, dtype=np.float32)
        for wi in range(NW):
            mm = ovf_w == wi
            n = int(mm.sum())
            s0 = int(ovf_tile_starts[wi]) * 128
            cols_stream[s0 : s0 + n] = ovf_row[mm].astype(np.int16)
            kk = np.arange(n)
            rloc_ovf[kk % 128, int(ovf_tile_starts[wi]) + kk // 128] = ovf_rloc[mm]

        rlocm = np.concatenate([rloc_reg, rloc_ovf], axis=1).astype(BF16)
        iota_rep = np.tile(
            np.arange(WIN, dtype=np.float32), (128, MCOLS)
        ).astype(BF16)

        in_maps.append(
            {
                "xt": XT,
                "w": Wb,
                "r8": np.ascontiguousarray(r8),
                "valm": np.ascontiguousarray(valm),
                "rlocm": np.ascontiguousarray(rlocm),
                "iota": np.ascontiguousarray(iota_rep),
                "cols_ovf": _wrap16(cols_stream, max(T_OVF_TOT, 1), pad=n_ovf_rows),
            }
        )
    meta = dict(
        ovf_fix=ovf_fix,
        n_ovf_rows=n_ovf_rows,
        T_ovf=[int(x) for x in T_ovf],
        ovf_tile_starts=[int(x) for x in ovf_tile_starts],
        T_OVF_TOT=T_OVF_TOT,
    )
    return in_maps, meta


def _build_nc(meta):
    f32 = mybir.dt.float32
    bf16 = mybir.dt.bfloat16
    fp8 = mybir.dt.float8e4
    i16 = mybir.dt.int16
    ovf_fix = meta["ovf_fix"]
    n_ovf_rows = meta["n_ovf_rows"]
    T_ovf = meta["T_ovf"]
    ovf_tile_starts = meta["ovf_tile_starts"]
    T_OVF_TOT = max(meta["T_OVF_TOT"], 1)

    nc = bacc.Bacc(
        get_trn_type() or "TRN2",
        target_bir_lowering=False,
        dynamic_dma_scratch_size=32768,
    )
    xt = nc.dram_tensor("xt", [F_IN, N_NODES], bf16, kind="ExternalInput")
    w_in = nc.dram_tensor("w", [F_IN, F_OUT], bf16, kind="ExternalInput")
    r8 = nc.dram_tensor("r8", [128, NT * SLOT_T], fp8, kind="ExternalInput")
    valm = nc.dram_tensor("valm", [128, NT * NCH], bf16, kind="ExternalInput")
    rlocm = nc.dram_tensor(
        "rlocm", [128, NW * MCOLS + T_OVF_TOT], bf16, kind="ExternalInput"
    )
    iota_in = nc.dram_tensor("iota", [128, MCOLS * WIN], bf16, kind="ExternalInput")
    cols_ovf = nc.dram_tensor(
        "cols_ovf", [128, T_OVF_TOT * 8], i16, kind="ExternalInput"
    )
    out = nc.dram_tensor("out", [SHARD, F_OUT], f32, kind="ExternalOutput")
    m1kind = "ExternalOutput" if DEBUG_M1 else "Internal"
    # M1REG[w][j][t][f]: row = (w*8 + j)*400 + t (t padded 391->400)
    m1reg = nc.dram_tensor("m1reg", [NW * KAP * NT_PAD, F_OUT], bf16, kind=m1kind)
    # +1: dedicated zero row for padded overflow-gather indices
    m1ovf = nc.dram_tensor("m1ovf", [n_ovf_rows + 1, 128], bf16, kind=m1kind)

    n_kc = F_IN // 128  # 2

    with tile.TileContext(nc) as tc, ExitStack() as ctx:
        const = ctx.enter_context(tc.tile_pool(name="const", bufs=1))
        xt_pool = ctx.enter_context(tc.tile_pool(name="xtp", bufs=3))
        r_pool = ctx.enter_context(tc.tile_pool(name="rp", bufs=3))
        ps_xw = ctx.enter_context(tc.tile_pool(name="ps_xw", bufs=2, space="PSUM"))
        ps_m1 = ctx.enter_context(tc.tile_pool(name="ps_m1", bufs=2, space="PSUM"))
        xw_pool = ctx.enter_context(tc.tile_pool(name="xwp", bufs=4))
        m1sb_pool = ctx.enter_context(tc.tile_pool(name="m1sb", bufs=4))
        mg_pool = ctx.enter_context(tc.tile_pool(name="mg", bufs=4))
        s_pool = ctx.enter_context(tc.tile_pool(name="sp", bufs=4))
        gath = ctx.enter_context(tc.tile_pool(name="gath", bufs=2))
        ps_out = ctx.enter_context(tc.tile_pool(name="ps_out", bufs=2, space="PSUM"))
        out_sb = ctx.enter_context(tc.tile_pool(name="osb", bufs=6))

        # resident constants
        w_t = []
        for k in range(n_kc):
            wt = const.tile([128, F_OUT], bf16, tag=f"w{k}")
            nc.sync.dma_start(out=wt[:], in_=w_in[k * 128 : (k + 1) * 128, :])
            w_t.append(wt)
        valm_t = const.tile([128, NT * NCH], bf16, tag="valm")
        nc.sync.dma_start(out=valm_t[:], in_=valm[:, :])
        rloc_t = const.tile([128, NW * MCOLS + T_OVF_TOT], bf16, tag="rloc")
        nc.sync.dma_start(out=rloc_t[:], in_=rlocm[:, :])
        iota_t = const.tile([128, MCOLS * WIN], bf16, tag="iota")
        nc.sync.dma_start(out=iota_t[:], in_=iota_in[:, :])
        cols_t = const.tile([128, T_OVF_TOT * 8], i16, tag="cols")
        nc.sync.dma_start(out=cols_t[:], in_=cols_ovf[:, :])
        # views: m1q[q][t][f] with q = slot chunk-major (c*128+p for c<3 maps
        # to q = w*8+j), merge window w = rows [w*3200, (w+1)*3200)
        m1q = m1reg[:, :].rearrange("(q t) f -> q t f", t=NT_PAD)
        m1ovf_v = m1ovf[: NT * ovf_fix, 0:F_OUT].rearrange(
            "(t k) f -> k t f", k=ovf_fix
        )  # [ovf_fix, NT, 64]

        # zero fills: overflow pad row + the t-pad columns (391..399) of m1reg
        zt = const.tile([128, 1764], bf16, tag="zt")
        nc.vector.memset(zt[:], 0)
        nc.sync.dma_start(out=m1ovf[n_ovf_rows : n_ovf_rows + 1, :], in_=zt[0:1, 0:128])
        padw = (NT_PAD - NT) * F_OUT  # 576
        for q0 in range(0, REG_SLOTS, 128):
            qn = min(128, REG_SLOTS - q0)
            nc.sync.dma_start(
                out=m1q[q0 : q0 + qn, NT:NT_PAD, :], in_=zt[:qn, :padw]
            )

        # ---- pass 1: fused XW + route-by-source, groups of G tiles ----
        xts = []
        s0 = 0
        rg0 = 0
        rsl = None
        for g0 in range(0, NT, G):
            gn = min(G, NT - g0)
            if g0 % (SLAB // 128) == 0:
                s0 = g0 * 128
                sl = min(SLAB, N_NODES - s0)
                xts = []
                for k in range(n_kc):
                    xtk = xt_pool.tile([128, SLAB], bf16, tag=f"xt{k}")
                    nc.sync.dma_start(
                        out=xtk[:, :sl],
                        in_=xt[k * 128 : (k + 1) * 128, s0 : s0 + sl],
                    )
                    xts.append(xtk)
            if g0 % RG == 0:
                rg0 = g0
                rgn = min(RG, NT - g0)
                rsl = r_pool.tile([128, RG * SLOT_T], fp8, tag="rsl")
                nc.sync.dma_start(
                    out=rsl[:, : rgn * SLOT_T],
                    in_=r8[:, rg0 * SLOT_T : (rg0 + rgn) * SLOT_T],
                )
            xwps = ps_xw.tile([128, G, F_OUT], f32, tag="xwps")
            last_m = 128
            for gi in range(gn):
                t = g0 + gi
                n0 = t * 128
                last_m = min(128, N_NODES - n0)
                for k in range(n_kc):
                    nc.tensor.matmul(
                        out=xwps[:last_m, gi, :],
                        lhsT=xts[k][:, n0 - s0 : n0 - s0 + last_m],
                        rhs=w_t[k][:],
                        start=(k == 0),
                        stop=(k == n_kc - 1),
                    )
            xw_sb = xw_pool.tile([128, G, F_OUT], bf16, tag="xw")
            if last_m < 128 or gn < G:
                nc.vector.memset(xw_sb[:], 0)
            if last_m < 128:  # ragged last tile: don't copy garbage PSUM rows
                if gn > 1:
                    nc.scalar.copy(out=xw_sb[:, : gn - 1, :], in_=xwps[:, : gn - 1, :])
                nc.scalar.copy(
                    out=xw_sb[:last_m, gn - 1, :], in_=xwps[:last_m, gn - 1, :]
                )
            else:
                nc.scalar.copy(out=xw_sb[:, :gn, :], in_=xwps[:, :gn, :])
            m1ps = ps_m1.tile([128, G * NCH, F_OUT], f32, tag="m1ps")
            for gi in range(gn):
                roff = (g0 + gi - rg0) * SLOT_T
                cw0 = 0
                for c in range(NCH):
                    cw = CHW[c]
                    nc.tensor.matmul(
                        out=m1ps[:cw, gi * NCH + c, :],
                        lhsT=rsl[:, roff + cw0 : roff + cw0 + cw],
                        rhs=xw_sb[:, gi, :],
                        start=True,
                        stop=True,
                    )
                    cw0 += cw
            m1sb = m1sb_pool.tile([128, G, NCH, F_OUT], bf16, tag="m1sb")
            nc.vector.tensor_tensor(
                out=m1sb[:, :gn, :, :].rearrange("p t c f -> p (t c) f"),
                in0=m1ps[:, : gn * NCH, :],
                in1=valm_t[:, g0 * NCH : (g0 + gn) * NCH].to_broadcast(
                    [128, gn * NCH, F_OUT]
                ),
                op=mybir.AluOpType.mult,
            )
            # 5 strided DMAs per group, alternating HWDGE queues by parity
            eng = nc.sync if (g0 // G) % 2 == 0 else nc.scalar
            for c in range(3):
                eng.dma_start(
                    out=m1q[c * 128 : (c + 1) * 128, g0 : g0 + gn, :],
                    in_=m1sb[:, :gn, c, :],
                )
            eng.dma_start(
                out=m1q[384:392, g0 : g0 + gn, :],
                in_=m1sb[0:8, :gn, 3, :],
            )
            eng.dma_start(
                out=m1ovf_v[:, g0 : g0 + gn, :],
                in_=m1sb[8 : 8 + ovf_fix, :gn, 3, :],
            )

        # ---- pass 2: merge by dest window + segment-sum ----
        gchunks = {}

        def ensure_gchunk(ti):
            ci = ti // CH_OVF
            if ci in gchunks:
                return gchunks[ci]
            cn = min(CH_OVF, T_OVF_TOT - ci * CH_OVF)
            g = gath.tile([128, CH_OVF, 128], bf16, tag="g")
            nc.gpsimd.dma_gather(
                out_ap=g[:, :cn, :],
                in_ap=m1ovf[:, :],
                idxs_ap=cols_t[:, ci * CH_OVF * 8 : (ci * CH_OVF + cn) * 8],
                num_idxs=cn * 128,
                num_idxs_reg=cn * 128,
                elem_size=128,
                single_packet=False,
            )
            gchunks[ci] = g
            return g

        for w in range(NW):
            mg = mg_pool.tile([128, MCOLS, F_OUT], bf16, tag="mg")
            weng = nc.scalar if w % 2 == 0 else nc.sync
            weng.dma_start(
                out=mg[:],
                in_=m1reg[
                    w * KAP * NT_PAD : (w + 1) * KAP * NT_PAD, :
                ].rearrange("(p a) f -> p a f", a=MCOLS),
            )
            S = s_pool.tile([128, MCOLS, WIN], bf16, tag="S")
            nc.vector.tensor_tensor(
                out=S[:],
                in0=iota_t[:].rearrange("p (a d) -> p a d", d=WIN),
                in1=rloc_t[:, w * MCOLS : (w + 1) * MCOLS].to_broadcast(
                    [128, MCOLS, WIN]
                ),
                op=mybir.AluOpType.is_equal,
            )
            nto = T_ovf[w]
            So = s_pool.tile([128, CH_OVF, WIN], bf16, tag="So")
            nc.vector.tensor_tensor(
                out=So[:, :nto, :],
                in0=iota_t[:, : nto * WIN].rearrange("p (a d) -> p a d", d=WIN),
                in1=rloc_t[
                    :, NW * MCOLS + ovf_tile_starts[w] : NW * MCOLS
                    + ovf_tile_starts[w] + nto
                ].to_broadcast([128, nto, WIN]),
                op=mybir.AluOpType.is_equal,
            )
            ops = ps_out.tile([128, F_OUT], f32, tag="ops")
            n_mm = MCOLS + nto
            mi = 0
            for a in range(MCOLS):
                nc.tensor.matmul(
                    out=ops[:],
                    lhsT=S[:, a, :],
                    rhs=mg[:, a, :],
                    start=(mi == 0),
                    stop=(mi == n_mm - 1),
                )
                mi += 1
            for k in range(nto):
                ti = ovf_tile_starts[w] + k
                g = ensure_gchunk(ti)
                nc.tensor.matmul(
                    out=ops[:],
                    lhsT=So[:, k, :],
                    rhs=g[:, ti - (ti // CH_OVF) * CH_OVF, 0:F_OUT],
                    start=(mi == 0),
                    stop=(mi == n_mm - 1),
                )
                mi += 1
            rows = min(WIN, SHARD - w * WIN)
            ot = out_sb.tile([128, F_OUT], f32, tag="ot")
            nc.scalar.copy(out=ot[:rows, :], in_=ops[:rows, :])
            weng.dma_start(out=out[w * WIN : w * WIN + rows, :], in_=ot[:rows, :])
    nc.compile()
    return nc


def kernel(X, W, edge_row, edge_col, edge_vals):
    global LAST_RESULTS, LAST_META, LAST_IN_MAPS
    X = np.asarray(X, dtype=np.float32)
    W = np.asarray(W, dtype=np.float32)
    edge_row = np.asarray(edge_row, dtype=np.int32)
    edge_col = np.asarray(edge_col, dtype=np.int32)
    edge_vals = np.asarray(edge_vals, dtype=np.float32)

    in_maps, meta = _prep(X, W, edge_row, edge_col, edge_vals)
    LAST_META, LAST_IN_MAPS = meta, in_maps
    nc = _build_nc(meta)
    trace = TRACE and _install_ntff_hook()
    res = run_bass_kernel_spmd(
        nc, in_maps, core_ids=list(range(N_CORES)), trace=trace
    )
    LAST_RESULTS = res
    out = np.concatenate([res.results[p]["out"] for p in range(N_CORES)], axis=0)
    return out.astype(np.float32)


# revision 3
# speedup vs baseline: 1.0307x; 1.0307x over previous
"""GCN layer on 8 TRN2 NeuronCores — two-pass kappa-dense shuffle.

out = segment_sum(edge_vals[:,None] * (X @ W)[edge_col], edge_row, N)

The v0 baseline gathered one 256B row per edge via gpsimd dma_gather; Q7
descriptor generation (~8ns/edge, serial) was 84% of its kernel time. Here
edges are routed in two passes with NO per-edge descriptors except a ~5%
overflow remainder:

Pass 1 (per 128-src tile t, in groups of G=4): XW_t computed on-chip (fused,
no XW roundtrip); a host-shipped 0/1 fp8 routing matrix R_t scatters each
edge's XW row via TensorE (M = R_t^T @ XW_t) into a dense per-(dest-window,
tile) kappa=8 slot layout; edge values fold in via one VectorE multiply per
group (cast+scale PSUM->SBUF). Rows land in DRAM with 5 strided DMAs per
group: M1REG[t][w*8+j] plus a fixed-capacity (28) per-tile overflow region.

Pass 2 (per dest-window w): ONE strided DMA pulls the window's [391*8, 64]
slot rows onto partitions; ONE batched VectorE is_equal builds all 25 one-hot
S chunks from rowloc metadata (dead slots rowloc=-1); TensorE accumulates
S^T @ rows into the window PSUM. Overflow edges ride the old dma_gather path
at 1/20 scale (~6k descriptors vs 115k).
"""

from contextlib import ExitStack

import ml_dtypes
import numpy as np

import concourse.bacc as bacc
import concourse.bass as bass
import concourse.mybir as mybir
import concourse.tile as tile
from concourse._compat import get_trn_type
from concourse.bass_utils import run_bass_kernel_spmd

N_NODES = 50000
N_EDGES = 800000
F_IN = 256
F_OUT = 64
N_CORES = 8
SHARD = N_NODES // N_CORES  # 6250
WIN = 128
NW = (SHARD + WIN - 1) // WIN  # 49
NT = (N_NODES + 127) // 128  # 391
KAP = 8
SLOT_T = 448  # slots per src tile: 392 regular + overflow + pad (3.5 chunks)
CHW = (128, 128, 128, 64)  # R chunk widths
NCH = len(CHW)
REG_SLOTS = NW * KAP  # 392
MCOLS = (NT * KAP + 127) // 128  # 25 merge chunks per window
NT_PAD = MCOLS * 16  # 400 tiles incl. 9 zero pad tiles for the merge AP
BF16 = ml_dtypes.bfloat16
FP8 = ml_dtypes.float8_e4m3

SLAB = 4096  # pass-1 node columns per XT slab DMA
RG = 16  # src tiles per R slab load
G = 4  # src tiles per pass-1 group (PSUM budget bound)
CH_OVF = 25  # overflow gather tiles per dma_gather call

TRACE = False
DEBUG_M1 = False
LAST_RESULTS = None
LAST_META = None
LAST_IN_MAPS = None


def _install_ntff_hook():
    import sys
    import types

    try:
        import antenv.axon_hooks  # noqa: F401

        return True
    except ImportError:
        pass
    try:
        import antenv
        from trn_agent_boot.trn_boot import _ntff_profile_via_ctypes

        mod = types.ModuleType("antenv.axon_hooks")
        mod._hook = None

        def set_axon_ntff_profile_hook(h):
            mod._hook = h

        def get_axon_ntff_profile_hook():
            return mod._hook

        mod.set_axon_ntff_profile_hook = set_axon_ntff_profile_hook
        mod.get_axon_ntff_profile_hook = get_axon_ntff_profile_hook
        sys.modules["antenv.axon_hooks"] = mod
        antenv.axon_hooks = mod
        hook = _ntff_profile_via_ctypes("/opt/axon/libaxon_pjrt.so")
        if hook is not None:
            set_axon_ntff_profile_hook(hook)
        return hook is not None
    except Exception as e:
        print(f"ntff hook install failed: {e}")
        return False


def _wrap16(stream_i16, n_tiles, pad=0):
    n = n_tiles * 128
    w = np.zeros((128, max(n // 16, 1)), dtype=np.int16)
    s = np.full(n, pad, dtype=np.int16)
    s[: len(stream_i16)] = stream_i16
    blk = s.reshape(n // 16, 16).T
    for g in range(8):
        w[g * 16 : (g + 1) * 16, :] = blk
    return w


def _prep(X, W, edge_row, edge_col, edge_vals):
    XT = np.ascontiguousarray(X.T).astype(BF16)
    Wb = np.ascontiguousarray(W).astype(BF16)

    core = edge_row // SHARD
    percore = []
    ovf_cnt = np.zeros((N_CORES, NT), dtype=np.int64)
    ovf_win_cnt = np.zeros((N_CORES, NW), dtype=np.int64)
    for p in range(N_CORES):
        m = core == p
        r = edge_row[m].astype(np.int64) - p * SHARD
        c = edge_col[m].astype(np.int64)
        v = edge_vals[m].astype(np.float64)
        t = c // 128
        sloc = c % 128
        w = r // WIN
        rloc = r % WIN
        order = np.lexsort((w, t))
        t, sloc, w, rloc, v = t[order], sloc[order], w[order], rloc[order], v[order]
        seg_id = t * NW + w
        cnts = np.bincount(seg_id, minlength=NT * NW)
        starts = np.concatenate([[0], np.cumsum(cnts)])
        rank = np.arange(len(t)) - starts[seg_id]
        is_ovf = rank >= KAP
        percore.append(
            dict(t=t, sloc=sloc, w=w, rloc=rloc, v=v, rank=rank, is_ovf=is_ovf)
        )
        ovf_cnt[p] = np.bincount(t[is_ovf], minlength=NT)
        ovf_win_cnt[p] = np.bincount(w[is_ovf], minlength=NW)

    ovf_fix = max(1, int(ovf_cnt.max()))  # fixed per-tile overflow capacity
    assert ovf_fix <= SLOT_T - REG_SLOTS - 8, ovf_fix
    n_ovf_rows = NT * ovf_fix
    assert n_ovf_rows < 32000, n_ovf_rows
    T_ovf = np.maximum(1, -(-ovf_win_cnt.max(axis=0) // 128))
    ovf_tile_starts = np.concatenate([[0], np.cumsum(T_ovf)])
    T_OVF_TOT = int(T_ovf.sum())
    assert T_OVF_TOT <= NW * CH_OVF

    in_maps = []
    for p in range(N_CORES):
        d = percore[p]
        t, sloc, w, rloc, v, rank, is_ovf = (
            d["t"], d["sloc"], d["w"], d["rloc"], d["v"], d["rank"], d["is_ovf"],
        )
        ne = len(t)
        slot = np.zeros(ne, dtype=np.int64)
        reg = ~is_ovf
        # slot map: w<48 -> (w//16)*128 + (w%16)*8 + j ; w=48 -> 384 + j.
        # Makes the M1[w][j][t] DRAM offset linear in the PSUM partition.
        wr, jr = w[reg], rank[reg]
        slot[reg] = np.where(
            wr < 48, (wr // 16) * 128 + (wr % 16) * 8 + jr, 384 + jr
        )
        ovf_idx_in_tile = np.zeros(ne, dtype=np.int64)
        for ti in np.unique(t[is_ovf]):
            mm = is_ovf & (t == ti)
            ovf_idx_in_tile[mm] = np.arange(mm.sum())
        slot[is_ovf] = 384 + 8 + ovf_idx_in_tile[is_ovf]

        r8 = np.zeros((128, NT * SLOT_T), dtype=FP8)
        valm = np.zeros((128, NT * NCH), dtype=BF16)
        gs = t * SLOT_T + slot
        r8[sloc, gs] = 1.0
        valm[slot % 128, t * NCH + slot // 128] = v.astype(BF16)

        # merge stream position within window: u = j*400 + t, landed
        # partition-major (partition u//25, col u%25) for fat DMA pieces
        rloc_reg = np.full((128, NW * MCOLS), -1.0, dtype=np.float32)
        u = rank[reg] * NT_PAD + t[reg]
        rloc_reg[u // MCOLS, w[reg] * MCOLS + u % MCOLS] = rloc[reg]

        # compact overflow rows at fixed per-tile stride
        ovf_row = t[is_ovf] * ovf_fix + ovf_idx_in_tile[is_ovf]
        ovf_w = w[is_ovf]
        ovf_rloc = rloc[is_ovf]
        o2 = np.argsort(ovf_w, kind="stable")
        ovf_row, ovf_w, ovf_rloc = ovf_row[o2], ovf_w[o2], ovf_rloc[o2]
        cols_stream = np.full(T_OVF_TOT * 128, n_ovf_rows, dtype=np.int16)
        rloc_ovf = np.full((128, T_OVF_TOT), -1.0, dtype=np.float32)
        for wi in range(NW):
            mm = ovf_w == wi
            n = int(mm.sum())
            s0 = int(ovf_tile_starts[wi]) * 128
            cols_stream[s0 : s0 + n] = ovf_row[mm].astype(np.int16)
            kk = np.arange(n)
            rloc_ovf[kk % 128, int(ovf_tile_starts[wi]) + kk // 128] = ovf_rloc[mm]

        rlocm = np.concatenate([rloc_reg, rloc_ovf], axis=1).astype(BF16)
        iota_rep = np.tile(
            np.arange(WIN, dtype=np.float32), (128, MCOLS)
        ).astype(BF16)

        in_maps.append(
            {
                "xt": XT,
                "w": Wb,
                "r8": np.ascontiguousarray(r8),
                "valm": np.ascontiguousarray(valm),
                "rlocm": np.ascontiguousarray(rlocm),
                "iota": np.ascontiguousarray(iota_rep),
                "cols_ovf": _wrap16(cols_stream, max(T_OVF_TOT, 1), pad=n_ovf_rows),
            }
        )
    meta = dict(
        ovf_fix=ovf_fix,
        n_ovf_rows=n_ovf_rows,
        T_ovf=[int(x) for x in T_ovf],
        ovf_tile_starts=[int(x) for x in ovf_tile_starts],
        T_OVF_TOT=T_OVF_TOT,
    )
    return in_maps, meta


def _build_nc(meta):
    f32 = mybir.dt.float32
    bf16 = mybir.dt.bfloat16
    fp8 = mybir.dt.float8e4
    i16 = mybir.dt.int16
    ovf_fix = meta["ovf_fix"]
    n_ovf_rows = meta["n_ovf_rows"]
    T_ovf = meta["T_ovf"]
    ovf_tile_starts = meta["ovf_tile_starts"]
    T_OVF_TOT = max(meta["T_OVF_TOT"], 1)

    nc = bacc.Bacc(
        get_trn_type() or "TRN2",
        target_bir_lowering=False,
        dynamic_dma_scratch_size=32768,
    )
    xt = nc.dram_tensor("xt", [F_IN, N_NODES], bf16, kind="ExternalInput")
    w_in = nc.dram_tensor("w", [F_IN, F_OUT], bf16, kind="ExternalInput")
    r8 = nc.dram_tensor("r8", [128, NT * SLOT_T], fp8, kind="ExternalInput")
    valm = nc.dram_tensor("valm", [128, NT * NCH], bf16, kind="ExternalInput")
    rlocm = nc.dram_tensor(
        "rlocm", [128, NW * MCOLS + T_OVF_TOT], bf16, kind="ExternalInput"
    )
    iota_in = nc.dram_tensor("iota", [128, MCOLS * WIN], bf16, kind="ExternalInput")
    cols_ovf = nc.dram_tensor(
        "cols_ovf", [128, T_OVF_TOT * 8], i16, kind="ExternalInput"
    )
    out = nc.dram_tensor("out", [SHARD, F_OUT], f32, kind="ExternalOutput")
    m1kind = "ExternalOutput" if DEBUG_M1 else "Internal"
    # M1REG[w][j][t][f]: row = (w*8 + j)*400 + t (t padded 391->400)
    m1reg = nc.dram_tensor("m1reg", [NW * KAP * NT_PAD, F_OUT], bf16, kind=m1kind)
    # +1: dedicated zero row for padded overflow-gather indices
    m1ovf = nc.dram_tensor("m1ovf", [n_ovf_rows + 1, 128], bf16, kind=m1kind)

    n_kc = F_IN // 128  # 2

    with tile.TileContext(nc) as tc, ExitStack() as ctx:
        const = ctx.enter_context(tc.tile_pool(name="const", bufs=1))
        xt_pool = ctx.enter_context(tc.tile_pool(name="xtp", bufs=3))
        r_pool = ctx.enter_context(tc.tile_pool(name="rp", bufs=3))
        ps_xw = ctx.enter_context(tc.tile_pool(name="ps_xw", bufs=2, space="PSUM"))
        ps_m1 = ctx.enter_context(tc.tile_pool(name="ps_m1", bufs=2, space="PSUM"))
        xw_pool = ctx.enter_context(tc.tile_pool(name="xwp", bufs=4))
        m1sb_pool = ctx.enter_context(tc.tile_pool(name="m1sb", bufs=4))
        mg_pool = ctx.enter_context(tc.tile_pool(name="mg", bufs=4))
        s_pool = ctx.enter_context(tc.tile_pool(name="sp", bufs=4))
        gath = ctx.enter_context(tc.tile_pool(name="gath", bufs=2))
        ps_out = ctx.enter_context(tc.tile_pool(name="ps_out", bufs=2, space="PSUM"))
        out_sb = ctx.enter_context(tc.tile_pool(name="osb", bufs=6))

        # resident constants
        w_t = []
        for k in range(n_kc):
            wt = const.tile([128, F_OUT], bf16, tag=f"w{k}")
            nc.sync.dma_start(out=wt[:], in_=w_in[k * 128 : (k + 1) * 128, :])
            w_t.append(wt)
        valm_t = const.tile([128, NT * NCH], bf16, tag="valm")
        nc.sync.dma_start(out=valm_t[:], in_=valm[:, :])
        rloc_t = const.tile([128, NW * MCOLS + T_OVF_TOT], bf16, tag="rloc")
        nc.sync.dma_start(out=rloc_t[:], in_=rlocm[:, :])
        iota_t = const.tile([128, MCOLS * WIN], bf16, tag="iota")
        nc.sync.dma_start(out=iota_t[:], in_=iota_in[:, :])
        cols_t = const.tile([128, T_OVF_TOT * 8], i16, tag="cols")
        nc.sync.dma_start(out=cols_t[:], in_=cols_ovf[:, :])
        # views: m1q[q][t][f] with q = slot chunk-major (c*128+p for c<3 maps
        # to q = w*8+j), merge window w = rows [w*3200, (w+1)*3200)
        m1q = m1reg[:, :].rearrange("(q t) f -> q t f", t=NT_PAD)
        m1ovf_v = m1ovf[: NT * ovf_fix, 0:F_OUT].rearrange(
            "(t k) f -> k t f", k=ovf_fix
        )  # [ovf_fix, NT, 64]

        # zero fills: overflow pad row + the t-pad columns (391..399) of m1reg
        zt = const.tile([128, 1764], bf16, tag="zt")
        nc.vector.memset(zt[:], 0)
        nc.sync.dma_start(out=m1ovf[n_ovf_rows : n_ovf_rows + 1, :], in_=zt[0:1, 0:128])
        padw = (NT_PAD - NT) * F_OUT  # 576
        for q0 in range(0, REG_SLOTS, 128):
            qn = min(128, REG_SLOTS - q0)
            nc.sync.dma_start(
                out=m1q[q0 : q0 + qn, NT:NT_PAD, :], in_=zt[:qn, :padw]
            )

        # ---- pass 1: fused XW + route-by-source, groups of G tiles ----
        xts = []
        s0 = 0
        rg0 = 0
        rsl = None
        for g0 in range(0, NT, G):
            gn = min(G, NT - g0)
            if g0 % (SLAB // 128) == 0:
                s0 = g0 * 128
                sl = min(SLAB, N_NODES - s0)
                xts = []
                for k in range(n_kc):
                    xtk = xt_pool.tile([128, SLAB], bf16, tag=f"xt{k}")
                    nc.sync.dma_start(
                        out=xtk[:, :sl],
                        in_=xt[k * 128 : (k + 1) * 128, s0 : s0 + sl],
                    )
                    xts.append(xtk)
            if g0 % RG == 0:
                rg0 = g0
                rgn = min(RG, NT - g0)
                rsl = r_pool.tile([128, RG * SLOT_T], fp8, tag="rsl")
                nc.sync.dma_start(
                    out=rsl[:, : rgn * SLOT_T],
                    in_=r8[:, rg0 * SLOT_T : (rg0 + rgn) * SLOT_T],
                )
            xwps = ps_xw.tile([128, G, F_OUT], f32, tag="xwps")
            last_m = 128
            for gi in range(gn):
                t = g0 + gi
                n0 = t * 128
                last_m = min(128, N_NODES - n0)
                for k in range(n_kc):
                    nc.tensor.matmul(
                        out=xwps[:last_m, gi, :],
                        lhsT=xts[k][:, n0 - s0 : n0 - s0 + last_m],
                        rhs=w_t[k][:],
                        start=(k == 0),
                        stop=(k == n_kc - 1),
                    )
            xw_sb = xw_pool.tile([128, G, F_OUT], bf16, tag="xw")
            if last_m < 128 or gn < G:
                nc.vector.memset(xw_sb[:], 0)
            if last_m < 128:  # ragged last tile: don't copy garbage PSUM rows
                if gn > 1:
                    nc.scalar.copy(out=xw_sb[:, : gn - 1, :], in_=xwps[:, : gn - 1, :])
                nc.scalar.copy(
                    out=xw_sb[:last_m, gn - 1, :], in_=xwps[:last_m, gn - 1, :]
                )
            else:
                nc.scalar.copy(out=xw_sb[:, :gn, :], in_=xwps[:, :gn, :])
            m1ps = ps_m1.tile([128, G * NCH, F_OUT], f32, tag="m1ps")
            for gi in range(gn):
                roff = (g0 + gi - rg0) * SLOT_T
                cw0 = 0
                for c in range(NCH):
                    cw = CHW[c]
                    nc.tensor.matmul(
                        out=m1ps[:cw, gi * NCH + c, :],
                        lhsT=rsl[:, roff + cw0 : roff + cw0 + cw],
                        rhs=xw_sb[:, gi, :],
                        start=True,
                        stop=True,
                    )
                    cw0 += cw
            m1sb = m1sb_pool.tile([128, G, NCH, F_OUT], bf16, tag="m1sb")
            nc.vector.tensor_tensor(
                out=m1sb[:, :gn, :, :].rearrange("p t c f -> p (t c) f"),
                in0=m1ps[:, : gn * NCH, :],
                in1=valm_t[:, g0 * NCH : (g0 + gn) * NCH].to_broadcast(
                    [128, gn * NCH, F_OUT]
                ),
                op=mybir.AluOpType.mult,
            )
            # 5 strided DMAs per group, split across both HWDGE queues
            for c in range(3):
                eng = nc.sync if c < 2 else nc.scalar
                eng.dma_start(
                    out=m1q[c * 128 : (c + 1) * 128, g0 : g0 + gn, :],
                    in_=m1sb[:, :gn, c, :],
                )
            nc.scalar.dma_start(
                out=m1q[384:392, g0 : g0 + gn, :],
                in_=m1sb[0:8, :gn, 3, :],
            )
            nc.scalar.dma_start(
                out=m1ovf_v[:, g0 : g0 + gn, :],
                in_=m1sb[8 : 8 + ovf_fix, :gn, 3, :],
            )

        # ---- pass 2: merge by dest window + segment-sum ----
        gchunks = {}

        def ensure_gchunk(ti):
            ci = ti // CH_OVF
            if ci in gchunks:
                return gchunks[ci]
            cn = min(CH_OVF, T_OVF_TOT - ci * CH_OVF)
            g = gath.tile([128, CH_OVF, 128], bf16, tag="g")
            nc.gpsimd.dma_gather(
                out_ap=g[:, :cn, :],
                in_ap=m1ovf[:, :],
                idxs_ap=cols_t[:, ci * CH_OVF * 8 : (ci * CH_OVF + cn) * 8],
                num_idxs=cn * 128,
                num_idxs_reg=cn * 128,
                elem_size=128,
                single_packet=False,
            )
            gchunks[ci] = g
            return g

        for ci in range(-(-T_OVF_TOT // CH_OVF)):
            ensure_gchunk(ci * CH_OVF)

        max_to = max(T_ovf)
        for w in range(NW):
            mg = mg_pool.tile([128, MCOLS, F_OUT], bf16, tag="mg")
            weng = nc.scalar if w % 2 == 0 else nc.sync
            weng.dma_start(
                out=mg[:],
                in_=m1reg[
                    w * KAP * NT_PAD : (w + 1) * KAP * NT_PAD, :
                ].rearrange("(p a) f -> p a f", a=MCOLS),
            )
            S = s_pool.tile([128, MCOLS, WIN], bf16, tag="S")
            nc.vector.tensor_tensor(
                out=S[:],
                in0=iota_t[:].rearrange("p (a d) -> p a d", d=WIN),
                in1=rloc_t[:, w * MCOLS : (w + 1) * MCOLS].to_broadcast(
                    [128, MCOLS, WIN]
                ),
                op=mybir.AluOpType.is_equal,
            )
            nto = T_ovf[w]
            So = s_pool.tile([128, max_to, WIN], bf16, tag="So")
            nc.vector.tensor_tensor(
                out=So[:, :nto, :],
                in0=iota_t[:, : nto * WIN].rearrange("p (a d) -> p a d", d=WIN),
                in1=rloc_t[
                    :, NW * MCOLS + ovf_tile_starts[w] : NW * MCOLS
                    + ovf_tile_starts[w] + nto
                ].to_broadcast([128, nto, WIN]),
                op=mybir.AluOpType.is_equal,
            )
            ops = ps_out.tile([128, F_OUT], f32, tag="ops")
            n_mm = MCOLS + nto
            mi = 0
            for a in range(MCOLS):
                nc.tensor.matmul(
                    out=ops[:],
                    lhsT=S[:, a, :],
                    rhs=mg[:, a, :],
                    start=(mi == 0),
                    stop=(mi == n_mm - 1),
                )
                mi += 1
            for k in range(nto):
                ti = ovf_tile_starts[w] + k
                g = ensure_gchunk(ti)
                nc.tensor.matmul(
                    out=ops[:],
                    lhsT=So[:, k, :],
                    rhs=g[:, ti - (ti // CH_OVF) * CH_OVF, 0:F_OUT],
                    start=(mi == 0),
                    stop=(mi == n_mm - 1),
                )
                mi += 1
            rows = min(WIN, SHARD - w * WIN)
            ot = out_sb.tile([128, F_OUT], f32, tag="ot")
            nc.scalar.copy(out=ot[:rows, :], in_=ops[:rows, :])
            weng.dma_start(out=out[w * WIN : w * WIN + rows, :], in_=ot[:rows, :])
    nc.compile()
    return nc


def kernel(X, W, edge_row, edge_col, edge_vals):
    global LAST_RESULTS, LAST_META, LAST_IN_MAPS
    X = np.asarray(X, dtype=np.float32)
    W = np.asarray(W, dtype=np.float32)
    edge_row = np.asarray(edge_row, dtype=np.int32)
    edge_col = np.asarray(edge_col, dtype=np.int32)
    edge_vals = np.asarray(edge_vals, dtype=np.float32)

    in_maps, meta = _prep(X, W, edge_row, edge_col, edge_vals)
    LAST_META, LAST_IN_MAPS = meta, in_maps
    nc = _build_nc(meta)
    trace = TRACE and _install_ntff_hook()
    res = run_bass_kernel_spmd(
        nc, in_maps, core_ids=list(range(N_CORES)), trace=trace
    )
    LAST_RESULTS = res
    out = np.concatenate([res.results[p]["out"] for p in range(N_CORES)], axis=0)
    return out.astype(np.float32)


# revision 4
# speedup vs baseline: 1.0772x; 1.0451x over previous
"""GCN layer on 8 TRN2 NeuronCores — two-pass kappa-dense shuffle.

out = segment_sum(edge_vals[:,None] * (X @ W)[edge_col], edge_row, N)

The v0 baseline gathered one 256B row per edge via gpsimd dma_gather; Q7
descriptor generation (~8ns/edge, serial) was 84% of its kernel time. Here
edges are routed in two passes with NO per-edge descriptors except a ~5%
overflow remainder:

Pass 1 (per 128-src tile t, in groups of G=4): XW_t computed on-chip (fused,
no XW roundtrip); a host-shipped 0/1 fp8 routing matrix R_t scatters each
edge's XW row via TensorE (M = R_t^T @ XW_t) into a dense per-(dest-window,
tile) kappa=8 slot layout; edge values fold in via one VectorE multiply per
group (cast+scale PSUM->SBUF). Rows land in DRAM with 5 strided DMAs per
group: M1REG[t][w*8+j] plus a fixed-capacity (28) per-tile overflow region.

Pass 2 (per dest-window w): ONE strided DMA pulls the window's [391*8, 64]
slot rows onto partitions; ONE batched VectorE is_equal builds all 25 one-hot
S chunks from rowloc metadata (dead slots rowloc=-1); TensorE accumulates
S^T @ rows into the window PSUM. Overflow edges ride the old dma_gather path
at 1/20 scale (~6k descriptors vs 115k).
"""

from contextlib import ExitStack

import ml_dtypes
import numpy as np

import concourse.bacc as bacc
import concourse.bass as bass
import concourse.mybir as mybir
import concourse.tile as tile
from concourse._compat import get_trn_type
from concourse.bass_utils import run_bass_kernel_spmd

N_NODES = 50000
N_EDGES = 800000
F_IN = 256
F_OUT = 64
N_CORES = 8
SHARD = N_NODES // N_CORES  # 6250
WIN = 128
NW = (SHARD + WIN - 1) // WIN  # 49
NT = (N_NODES + 127) // 128  # 391
KAP = 8
SLOT_T = 448  # slots per src tile: 392 regular + overflow + pad (3.5 chunks)
CHW = (128, 128, 128, 64)  # R chunk widths
NCH = len(CHW)
REG_SLOTS = NW * KAP  # 392
MCOLS = (NT * KAP + 127) // 128  # 25 merge chunks per window
NT_PAD = MCOLS * 16  # 400 tiles incl. 9 zero pad tiles for the merge AP
# source split for phase overlap: A = tiles [0,192), B = [192, 400)
TSPL = 192
TB = NT_PAD - TSPL  # 208
ACOLS = KAP * TSPL // 128  # 12 merge cols from half A
BCOLS = KAP * TB // 128  # 13 merge cols from half B
BF16 = ml_dtypes.bfloat16
FP8 = ml_dtypes.float8_e4m3

SLAB = 4096  # pass-1 node columns per XT slab DMA
RG = 16  # src tiles per R slab load
G = 4  # src tiles per pass-1 group (PSUM budget bound)
CH_OVF = 25  # overflow gather tiles per dma_gather call

TRACE = False
DEBUG_M1 = False
LAST_RESULTS = None
LAST_META = None
LAST_IN_MAPS = None


def _install_ntff_hook():
    import sys
    import types

    try:
        import antenv.axon_hooks  # noqa: F401

        return True
    except ImportError:
        pass
    try:
        import antenv
        from trn_agent_boot.trn_boot import _ntff_profile_via_ctypes

        mod = types.ModuleType("antenv.axon_hooks")
        mod._hook = None

        def set_axon_ntff_profile_hook(h):
            mod._hook = h

        def get_axon_ntff_profile_hook():
            return mod._hook

        mod.set_axon_ntff_profile_hook = set_axon_ntff_profile_hook
        mod.get_axon_ntff_profile_hook = get_axon_ntff_profile_hook
        sys.modules["antenv.axon_hooks"] = mod
        antenv.axon_hooks = mod
        hook = _ntff_profile_via_ctypes("/opt/axon/libaxon_pjrt.so")
        if hook is not None:
            set_axon_ntff_profile_hook(hook)
        return hook is not None
    except Exception as e:
        print(f"ntff hook install failed: {e}")
        return False


def _wrap16(stream_i16, n_tiles, pad=0):
    n = n_tiles * 128
    w = np.zeros((128, max(n // 16, 1)), dtype=np.int16)
    s = np.full(n, pad, dtype=np.int16)
    s[: len(stream_i16)] = stream_i16
    blk = s.reshape(n // 16, 16).T
    for g in range(8):
        w[g * 16 : (g + 1) * 16, :] = blk
    return w


def _prep(X, W, edge_row, edge_col, edge_vals):
    XT = np.ascontiguousarray(X.T).astype(BF16)
    Wb = np.ascontiguousarray(W).astype(BF16)

    core = edge_row // SHARD
    percore = []
    ovf_cnt = np.zeros((N_CORES, NT), dtype=np.int64)
    ovf_win_cnt = np.zeros((N_CORES, NW), dtype=np.int64)
    for p in range(N_CORES):
        m = core == p
        r = edge_row[m].astype(np.int64) - p * SHARD
        c = edge_col[m].astype(np.int64)
        v = edge_vals[m].astype(np.float64)
        t = c // 128
        sloc = c % 128
        w = r // WIN
        rloc = r % WIN
        order = np.lexsort((w, t))
        t, sloc, w, rloc, v = t[order], sloc[order], w[order], rloc[order], v[order]
        seg_id = t * NW + w
        cnts = np.bincount(seg_id, minlength=NT * NW)
        starts = np.concatenate([[0], np.cumsum(cnts)])
        rank = np.arange(len(t)) - starts[seg_id]
        is_ovf = rank >= KAP
        percore.append(
            dict(t=t, sloc=sloc, w=w, rloc=rloc, v=v, rank=rank, is_ovf=is_ovf)
        )
        ovf_cnt[p] = np.bincount(t[is_ovf], minlength=NT)
        ovf_win_cnt[p] = np.bincount(w[is_ovf], minlength=NW)

    ovf_fix = max(1, int(ovf_cnt.max()))  # fixed per-tile overflow capacity
    assert ovf_fix <= SLOT_T - REG_SLOTS - 8, ovf_fix
    n_ovf_rows = NT * ovf_fix
    assert n_ovf_rows < 32000, n_ovf_rows
    T_ovf = np.maximum(1, -(-ovf_win_cnt.max(axis=0) // 128))
    ovf_tile_starts = np.concatenate([[0], np.cumsum(T_ovf)])
    T_OVF_TOT = int(T_ovf.sum())
    assert T_OVF_TOT <= NW * CH_OVF

    in_maps = []
    for p in range(N_CORES):
        d = percore[p]
        t, sloc, w, rloc, v, rank, is_ovf = (
            d["t"], d["sloc"], d["w"], d["rloc"], d["v"], d["rank"], d["is_ovf"],
        )
        ne = len(t)
        slot = np.zeros(ne, dtype=np.int64)
        reg = ~is_ovf
        # slot map: w<48 -> (w//16)*128 + (w%16)*8 + j ; w=48 -> 384 + j.
        # Makes the M1[w][j][t] DRAM offset linear in the PSUM partition.
        wr, jr = w[reg], rank[reg]
        slot[reg] = np.where(
            wr < 48, (wr // 16) * 128 + (wr % 16) * 8 + jr, 384 + jr
        )
        ovf_idx_in_tile = np.zeros(ne, dtype=np.int64)
        for ti in np.unique(t[is_ovf]):
            mm = is_ovf & (t == ti)
            ovf_idx_in_tile[mm] = np.arange(mm.sum())
        slot[is_ovf] = 384 + 8 + ovf_idx_in_tile[is_ovf]

        r8 = np.zeros((128, NT * SLOT_T), dtype=FP8)
        valm = np.zeros((128, NT * NCH), dtype=BF16)
        gs = t * SLOT_T + slot
        r8[sloc, gs] = 1.0
        valm[slot % 128, t * NCH + slot // 128] = v.astype(BF16)

        # merge stream position within window, split by source half for
        # pass-1/pass-2 overlap: A = t<192 (12 cols), B = t>=192 (13 cols),
        # each landed partition-major (fat DMA pieces)
        rloc_reg = np.full((128, NW * MCOLS), -1.0, dtype=np.float32)
        tr, jr2 = t[reg], rank[reg]
        inA = tr < TSPL
        uA = jr2 * TSPL + tr
        uB = jr2 * TB + (tr - TSPL)
        pa = np.where(inA, uA // ACOLS, uB // BCOLS)
        ca = np.where(inA, uA % ACOLS, ACOLS + uB % BCOLS)
        rloc_reg[pa, w[reg] * MCOLS + ca] = rloc[reg]

        # compact overflow rows at fixed per-tile stride
        ovf_row = t[is_ovf] * ovf_fix + ovf_idx_in_tile[is_ovf]
        ovf_w = w[is_ovf]
        ovf_rloc = rloc[is_ovf]
        o2 = np.argsort(ovf_w, kind="stable")
        ovf_row, ovf_w, ovf_rloc = ovf_row[o2], ovf_w[o2], ovf_rloc[o2]
        cols_stream = np.full(T_OVF_TOT * 128, n_ovf_rows, dtype=np.int16)
        rloc_ovf = np.full((128, T_OVF_TOT), -1.0, dtype=np.float32)
        for wi in range(NW):
            mm = ovf_w == wi
            n = int(mm.sum())
            s0 = int(ovf_tile_starts[wi]) * 128
            cols_stream[s0 : s0 + n] = ovf_row[mm].astype(np.int16)
            kk = np.arange(n)
            rloc_ovf[kk % 128, int(ovf_tile_starts[wi]) + kk // 128] = ovf_rloc[mm]

        rlocm = np.concatenate([rloc_reg, rloc_ovf], axis=1).astype(BF16)
        iota_rep = np.tile(
            np.arange(WIN, dtype=np.float32), (128, MCOLS)
        ).astype(BF16)

        in_maps.append(
            {
                "xt": XT,
                "w": Wb,
                "r8": np.ascontiguousarray(r8),
                "valm": np.ascontiguousarray(valm),
                "rlocm": np.ascontiguousarray(rlocm),
                "iota": np.ascontiguousarray(iota_rep),
                "cols_ovf": _wrap16(cols_stream, max(T_OVF_TOT, 1), pad=n_ovf_rows),
            }
        )
    meta = dict(
        ovf_fix=ovf_fix,
        n_ovf_rows=n_ovf_rows,
        T_ovf=[int(x) for x in T_ovf],
        ovf_tile_starts=[int(x) for x in ovf_tile_starts],
        T_OVF_TOT=T_OVF_TOT,
    )
    return in_maps, meta


def _build_nc(meta):
    f32 = mybir.dt.float32
    bf16 = mybir.dt.bfloat16
    fp8 = mybir.dt.float8e4
    i16 = mybir.dt.int16
    ovf_fix = meta["ovf_fix"]
    n_ovf_rows = meta["n_ovf_rows"]
    T_ovf = meta["T_ovf"]
    ovf_tile_starts = meta["ovf_tile_starts"]
    T_OVF_TOT = max(meta["T_OVF_TOT"], 1)

    nc = bacc.Bacc(
        get_trn_type() or "TRN2",
        target_bir_lowering=False,
        dynamic_dma_scratch_size=32768,
    )
    xt = nc.dram_tensor("xt", [F_IN, N_NODES], bf16, kind="ExternalInput")
    w_in = nc.dram_tensor("w", [F_IN, F_OUT], bf16, kind="ExternalInput")
    r8 = nc.dram_tensor("r8", [128, NT * SLOT_T], fp8, kind="ExternalInput")
    valm = nc.dram_tensor("valm", [128, NT * NCH], bf16, kind="ExternalInput")
    rlocm = nc.dram_tensor(
        "rlocm", [128, NW * MCOLS + T_OVF_TOT], bf16, kind="ExternalInput"
    )
    iota_in = nc.dram_tensor("iota", [128, MCOLS * WIN], bf16, kind="ExternalInput")
    cols_ovf = nc.dram_tensor(
        "cols_ovf", [128, T_OVF_TOT * 8], i16, kind="ExternalInput"
    )
    out = nc.dram_tensor("out", [SHARD, F_OUT], f32, kind="ExternalOutput")
    m1kind = "ExternalOutput" if DEBUG_M1 else "Internal"
    # M1 split by source half for overlap: A rows (w*8+j)*192 + t,
    # B rows (w*8+j)*208 + (t-192) (t padded 391->400)
    m1regA = nc.dram_tensor("m1regA", [NW * KAP * TSPL, F_OUT], bf16, kind=m1kind)
    m1regB = nc.dram_tensor("m1regB", [NW * KAP * TB, F_OUT], bf16, kind=m1kind)
    # +1: dedicated zero row for padded overflow-gather indices
    m1ovf = nc.dram_tensor("m1ovf", [n_ovf_rows + 1, 128], bf16, kind=m1kind)

    n_kc = F_IN // 128  # 2

    with tile.TileContext(nc) as tc, ExitStack() as ctx:
        const = ctx.enter_context(tc.tile_pool(name="const", bufs=1))
        xt_pool = ctx.enter_context(tc.tile_pool(name="xtp", bufs=3))
        r_pool = ctx.enter_context(tc.tile_pool(name="rp", bufs=3))
        ps_xw = ctx.enter_context(tc.tile_pool(name="ps_xw", bufs=2, space="PSUM"))
        ps_m1 = ctx.enter_context(tc.tile_pool(name="ps_m1", bufs=2, space="PSUM"))
        xw_pool = ctx.enter_context(tc.tile_pool(name="xwp", bufs=4))
        m1sb_pool = ctx.enter_context(tc.tile_pool(name="m1sb", bufs=4))
        mg_pool = ctx.enter_context(tc.tile_pool(name="mg", bufs=4))
        s_pool = ctx.enter_context(tc.tile_pool(name="sp", bufs=4))
        gath = ctx.enter_context(tc.tile_pool(name="gath", bufs=2))
        ps_out = ctx.enter_context(tc.tile_pool(name="ps_out", bufs=2, space="PSUM"))
        out_sb = ctx.enter_context(tc.tile_pool(name="osb", bufs=6))

        # resident constants
        w_t = []
        for k in range(n_kc):
            wt = const.tile([128, F_OUT], bf16, tag=f"w{k}")
            nc.sync.dma_start(out=wt[:], in_=w_in[k * 128 : (k + 1) * 128, :])
            w_t.append(wt)
        valm_t = const.tile([128, NT * NCH], bf16, tag="valm")
        nc.sync.dma_start(out=valm_t[:], in_=valm[:, :])
        rloc_t = const.tile([128, NW * MCOLS + T_OVF_TOT], bf16, tag="rloc")
        nc.sync.dma_start(out=rloc_t[:], in_=rlocm[:, :])
        iota_t = const.tile([128, MCOLS * WIN], bf16, tag="iota")
        nc.sync.dma_start(out=iota_t[:], in_=iota_in[:, :])
        cols_t = const.tile([128, T_OVF_TOT * 8], i16, tag="cols")
        nc.sync.dma_start(out=cols_t[:], in_=cols_ovf[:, :])
        # views: m1q*[q][t][f] with q = slot chunk-major (c*128+p maps to
        # q = w*8+j)
        m1qA = m1regA[:, :].rearrange("(q t) f -> q t f", t=TSPL)
        m1qB = m1regB[:, :].rearrange("(q t) f -> q t f", t=TB)
        m1ovf_v = m1ovf[: NT * ovf_fix, 0:F_OUT].rearrange(
            "(t k) f -> k t f", k=ovf_fix
        )  # [ovf_fix, NT, 64]
        # session-A results, one [128, 64] fp32 slab per window
        acc = const.tile([128, NW, F_OUT], f32, tag="acc")

        # zero fills: overflow pad row + the t-pad columns (391..399) in B
        zt = const.tile([128, 1764], bf16, tag="zt")
        nc.vector.memset(zt[:], 0)
        nc.sync.dma_start(out=m1ovf[n_ovf_rows : n_ovf_rows + 1, :], in_=zt[0:1, 0:128])
        padw = (NT_PAD - NT) * F_OUT  # 576
        for q0 in range(0, REG_SLOTS, 128):
            qn = min(128, REG_SLOTS - q0)
            nc.sync.dma_start(
                out=m1qB[q0 : q0 + qn, NT - TSPL : TB, :], in_=zt[:qn, :padw]
            )

        # session A: one window's half-A merge + matmuls -> acc (interleaved
        # into the back half of pass 1)
        def emit_A(w):
            mgA = mg_pool.tile([128, ACOLS, F_OUT], bf16, tag="mgA")
            weng = nc.scalar if w % 2 == 0 else nc.sync
            weng.dma_start(
                out=mgA[:],
                in_=m1regA[w * KAP * TSPL : (w + 1) * KAP * TSPL, :].rearrange(
                    "(p a) f -> p a f", a=ACOLS
                ),
            )
            SA = s_pool.tile([128, ACOLS, WIN], bf16, tag="SA")
            nc.vector.tensor_tensor(
                out=SA[:],
                in0=iota_t[:, : ACOLS * WIN].rearrange("p (a d) -> p a d", d=WIN),
                in1=rloc_t[:, w * MCOLS : w * MCOLS + ACOLS].to_broadcast(
                    [128, ACOLS, WIN]
                ),
                op=mybir.AluOpType.is_equal,
            )
            opsa = ps_out.tile([128, F_OUT], f32, tag="ops")
            for a in range(ACOLS):
                nc.tensor.matmul(
                    out=opsa[:],
                    lhsT=SA[:, a, :],
                    rhs=mgA[:, a, :],
                    start=(a == 0),
                    stop=(a == ACOLS - 1),
                )
            nc.scalar.copy(out=acc[:, w, :], in_=opsa[:])

        # ---- pass 1: fused XW + route-by-source, groups of G tiles ----
        xts = []
        s0 = 0
        rg0 = 0
        rsl = None
        for g0 in range(0, NT, G):
            gn = min(G, NT - g0)
            if g0 % (SLAB // 128) == 0:
                s0 = g0 * 128
                sl = min(SLAB, N_NODES - s0)
                xts = []
                for k in range(n_kc):
                    xtk = xt_pool.tile([128, SLAB], bf16, tag=f"xt{k}")
                    nc.sync.dma_start(
                        out=xtk[:, :sl],
                        in_=xt[k * 128 : (k + 1) * 128, s0 : s0 + sl],
                    )
                    xts.append(xtk)
            if g0 % RG == 0:
                rg0 = g0
                rgn = min(RG, NT - g0)
                rsl = r_pool.tile([128, RG * SLOT_T], fp8, tag="rsl")
                nc.sync.dma_start(
                    out=rsl[:, : rgn * SLOT_T],
                    in_=r8[:, rg0 * SLOT_T : (rg0 + rgn) * SLOT_T],
                )
            xwps = ps_xw.tile([128, G, F_OUT], f32, tag="xwps")
            last_m = 128
            for gi in range(gn):
                t = g0 + gi
                n0 = t * 128
                last_m = min(128, N_NODES - n0)
                for k in range(n_kc):
                    nc.tensor.matmul(
                        out=xwps[:last_m, gi, :],
                        lhsT=xts[k][:, n0 - s0 : n0 - s0 + last_m],
                        rhs=w_t[k][:],
                        start=(k == 0),
                        stop=(k == n_kc - 1),
                    )
            xw_sb = xw_pool.tile([128, G, F_OUT], bf16, tag="xw")
            if last_m < 128 or gn < G:
                nc.vector.memset(xw_sb[:], 0)
            if last_m < 128:  # ragged last tile: don't copy garbage PSUM rows
                if gn > 1:
                    nc.scalar.copy(out=xw_sb[:, : gn - 1, :], in_=xwps[:, : gn - 1, :])
                nc.scalar.copy(
                    out=xw_sb[:last_m, gn - 1, :], in_=xwps[:last_m, gn - 1, :]
                )
            else:
                nc.scalar.copy(out=xw_sb[:, :gn, :], in_=xwps[:, :gn, :])
            m1ps = ps_m1.tile([128, G * NCH, F_OUT], f32, tag="m1ps")
            for gi in range(gn):
                roff = (g0 + gi - rg0) * SLOT_T
                cw0 = 0
                for c in range(NCH):
                    cw = CHW[c]
                    nc.tensor.matmul(
                        out=m1ps[:cw, gi * NCH + c, :],
                        lhsT=rsl[:, roff + cw0 : roff + cw0 + cw],
                        rhs=xw_sb[:, gi, :],
                        start=True,
                        stop=True,
                    )
                    cw0 += cw
            m1sb = m1sb_pool.tile([128, G, NCH, F_OUT], bf16, tag="m1sb")
            nc.vector.tensor_tensor(
                out=m1sb[:, :gn, :, :].rearrange("p t c f -> p (t c) f"),
                in0=m1ps[:, : gn * NCH, :],
                in1=valm_t[:, g0 * NCH : (g0 + gn) * NCH].to_broadcast(
                    [128, gn * NCH, F_OUT]
                ),
                op=mybir.AluOpType.mult,
            )
            # 5 strided DMAs per group, split across both HWDGE queues
            m1q, tg0 = (m1qA, g0) if g0 < TSPL else (m1qB, g0 - TSPL)
            for c in range(3):
                eng = nc.sync if c < 2 else nc.scalar
                eng.dma_start(
                    out=m1q[c * 128 : (c + 1) * 128, tg0 : tg0 + gn, :],
                    in_=m1sb[:, :gn, c, :],
                )
            nc.scalar.dma_start(
                out=m1q[384:392, tg0 : tg0 + gn, :],
                in_=m1sb[0:8, :gn, 3, :],
            )
            nc.scalar.dma_start(
                out=m1ovf_v[:, g0 : g0 + gn, :],
                in_=m1sb[8 : 8 + ovf_fix, :gn, 3, :],
            )
            # overlap: emit one session-A window per group once half A is down
            gidx = g0 // G
            if gidx >= TSPL // G and gidx - TSPL // G < NW:
                emit_A(gidx - TSPL // G)

        # ---- pass 2: merge by dest window + segment-sum ----
        gchunks = {}

        def ensure_gchunk(ti):
            ci = ti // CH_OVF
            if ci in gchunks:
                return gchunks[ci]
            cn = min(CH_OVF, T_OVF_TOT - ci * CH_OVF)
            g = gath.tile([128, CH_OVF, 128], bf16, tag="g")
            nc.gpsimd.dma_gather(
                out_ap=g[:, :cn, :],
                in_ap=m1ovf[:, :],
                idxs_ap=cols_t[:, ci * CH_OVF * 8 : (ci * CH_OVF + cn) * 8],
                num_idxs=cn * 128,
                num_idxs_reg=cn * 128,
                elem_size=128,
                single_packet=False,
            )
            gchunks[ci] = g
            return g

        for ci in range(-(-T_OVF_TOT // CH_OVF)):
            ensure_gchunk(ci * CH_OVF)

        max_to = max(T_ovf)
        for w in range(NW):
            mg = mg_pool.tile([128, BCOLS, F_OUT], bf16, tag="mgB")
            weng = nc.scalar if w % 2 == 0 else nc.sync
            weng.dma_start(
                out=mg[:],
                in_=m1regB[
                    w * KAP * TB : (w + 1) * KAP * TB, :
                ].rearrange("(p a) f -> p a f", a=BCOLS),
            )
            S = s_pool.tile([128, BCOLS, WIN], bf16, tag="SB")
            nc.vector.tensor_tensor(
                out=S[:],
                in0=iota_t[:, : BCOLS * WIN].rearrange("p (a d) -> p a d", d=WIN),
                in1=rloc_t[
                    :, w * MCOLS + ACOLS : (w + 1) * MCOLS
                ].to_broadcast([128, BCOLS, WIN]),
                op=mybir.AluOpType.is_equal,
            )
            nto = T_ovf[w]
            So = s_pool.tile([128, max_to, WIN], bf16, tag="So")
            nc.vector.tensor_tensor(
                out=So[:, :nto, :],
                in0=iota_t[:, : nto * WIN].rearrange("p (a d) -> p a d", d=WIN),
                in1=rloc_t[
                    :, NW * MCOLS + ovf_tile_starts[w] : NW * MCOLS
                    + ovf_tile_starts[w] + nto
                ].to_broadcast([128, nto, WIN]),
                op=mybir.AluOpType.is_equal,
            )
            ops = ps_out.tile([128, F_OUT], f32, tag="ops")
            n_mm = BCOLS + nto
            mi = 0
            for a in range(BCOLS):
                nc.tensor.matmul(
                    out=ops[:],
                    lhsT=S[:, a, :],
                    rhs=mg[:, a, :],
                    start=(mi == 0),
                    stop=(mi == n_mm - 1),
                )
                mi += 1
            for k in range(nto):
                ti = ovf_tile_starts[w] + k
                g = ensure_gchunk(ti)
                nc.tensor.matmul(
                    out=ops[:],
                    lhsT=So[:, k, :],
                    rhs=g[:, ti - (ti // CH_OVF) * CH_OVF, 0:F_OUT],
                    start=(mi == 0),
                    stop=(mi == n_mm - 1),
                )
                mi += 1
            rows = min(WIN, SHARD - w * WIN)
            ot = out_sb.tile([128, F_OUT], f32, tag="ot")
            nc.vector.tensor_tensor(
                out=ot[:rows, :],
                in0=ops[:rows, :],
                in1=acc[:rows, w, :],
                op=mybir.AluOpType.add,
            )
            weng.dma_start(out=out[w * WIN : w * WIN + rows, :], in_=ot[:rows, :])
    nc.compile()
    return nc


def kernel(X, W, edge_row, edge_col, edge_vals):
    global LAST_RESULTS, LAST_META, LAST_IN_MAPS
    X = np.asarray(X, dtype=np.float32)
    W = np.asarray(W, dtype=np.float32)
    edge_row = np.asarray(edge_row, dtype=np.int32)
    edge_col = np.asarray(edge_col, dtype=np.int32)
    edge_vals = np.asarray(edge_vals, dtype=np.float32)

    in_maps, meta = _prep(X, W, edge_row, edge_col, edge_vals)
    LAST_META, LAST_IN_MAPS = meta, in_maps
    nc = _build_nc(meta)
    trace = TRACE and _install_ntff_hook()
    res = run_bass_kernel_spmd(
        nc, in_maps, core_ids=list(range(N_CORES)), trace=trace
    )
    LAST_RESULTS = res
    out = np.concatenate([res.results[p]["out"] for p in range(N_CORES)], axis=0)
    return out.astype(np.float32)
